# revision 17
# baseline (speedup 1.0000x reference)
"""GATNE model on 8 Trainium2 NeuronCores via Bass/Tile — v3.

v3 replaces both descriptor-limited dma_gathers with dense-DMA + PE matmuls
(measured: dma_gather ~2.4-3.9 ns/row vs dense DMA ~360+ GB/s; PE matmul
supports MIXED operand dtypes — fp8e4 count matrices x bf16 tables — at
~0.47 ns/row with N=64, verified exact for integer counts <= 16):

Phase 1 (SPMD, data-parallel over B, samples sorted by type so tiles are
type-homogeneous): the neighbor gather-sum becomes, per (tile, type), a
K=2560-node one-hot COUNT matmul: nte[b,u] = sum_s OH[s-block,b]^T @
ttab_t[s-block,u], counts shipped from host as fp8e4 (exact: counts<=10),
ttab resident in SBUF as bf16. Attention/delta/normalize as before.
Phase 2 (SPMD, sharded by 256-segment blocks): segment-sum becomes a dense
matmul: acc[seg,e] = sum_k MT[k-block,seg]^T @ lne[k-block,e] with MT the
fp8e4 per-(sample-slot, segment) count matrix and lne the bf16 embedding
table streamed block-wise. Mean+normalize, FC, l2norm as before.
"""
import contextlib
import sys

sys.path.insert(0, "/opt/trn_rl_repo")

import numpy as np
import ml_dtypes

import concourse.bacc as bacc
import concourse.bass as bass
import concourse.mybir as mybir
import concourse.tile as tile
from concourse.bass_utils import run_bass_kernel_spmd
from concourse.masks import make_identity

F32 = mybir.dt.float32
BF16 = mybir.dt.bfloat16
FP8 = mybir.dt.float8e4
I16 = mybir.dt.int16
AF = mybir.ActivationFunctionType
ALU = mybir.AluOpType
NPBF16 = ml_dtypes.bfloat16
NPFP8 = ml_dtypes.float8_e4m3

N_CORES = 8
NUM_NODES = 2500
T = 4
EMB = 256
U = 64
DIM_A = 32
EMBED_SIZE = 512
NEIGH = 10
B = NUM_NODES * T
NUM_SMS = 64
MAX_REGION = 32
NSEG = NUM_SMS * MAX_REGION  # 2048
SEG_PER_CORE = NSEG // N_CORES  # 256
E_TOTAL = 131072
P = 128

NQ = 4  # SWDGE queues (node_embeddings gather only)
NBLK = 20  # 2560-padded node blocks of 128
P2_CHUNK = 10  # K-blocks per phase-2 stream chunk


def _wrap16(flat):
    """dma_gather index layout: idx i -> partition i%16, column i//16,
    replicated across the 8 partition groups (one per Q7 core)."""
    n = flat.shape[0]
    assert n % 16 == 0
    a = flat.reshape(n // 16, 16).T.astype(np.int16)  # [16, n//16]
    return np.tile(a, (8, 1))  # [128, n//16]


# ---------------------------------------------------------------------------
# walrus post-pass: CoreV3 codegen rejects >1 sem wait on a TPB_CTRL (Drain);
# split the excess onto injected wait-only drains placed just before.
# ---------------------------------------------------------------------------
def _split_drain_waits(nc, max_waits=1):
    for bb in nc.main_func.blocks:
        out = []
        for ins in bb.instructions:
            si = ins.sync_info
            if (
                type(ins).__name__ == "InstDrain"
                and si is not None
                and si.on_wait is not None
                and len(si.on_wait) > max_waits
            ):
                waits = list(si.on_wait)
                extra, keep = waits[:-max_waits], waits[-max_waits:]
                for i in range(0, len(extra), max_waits):
                    d = mybir.InstDrain(
                        name=nc.get_next_instruction_name(),
                        ins=[],
                        outs=[],
                        bass_is_fusable=False,
                    )
                    d.engine = ins.engine
                    d.sync_info = mybir.SyncInfo(
                        on_wait=extra[i : i + max_waits], on_update=[]
                    )
                    out.append(d)
                ins.sync_info = mybir.SyncInfo(
                    on_wait=keep, on_update=list(si.on_update or [])
                )
            out.append(ins)
        bb.instructions = out
    return nc


# ---------------------------------------------------------------------------
# Phase-1 program builder
# ---------------------------------------------------------------------------
def build_phase1(TPC, reps=1, ohbufs=4, mode='full'):
    OHC = T * NBLK * P  # one-hot columns per tile (4*20*128)
    nc = bacc.Bacc("TRN2", debug=False, num_swdge_queues=NQ)
    ttab = nc.dram_tensor("ttab", [P, T * NBLK * U], BF16, kind="ExternalInput")
    oh = nc.dram_tensor("oh", [P, TPC * OHC], FP8, kind="ExternalInput")
    ntab = nc.dram_tensor("ntab", [NUM_NODES, EMB], F32, kind="ExternalInput")
    neidx = nc.dram_tensor("neidx", [P, TPC * 8], I16, kind="ExternalInput")
    s1w = nc.dram_tensor("s1w", [U, TPC * DIM_A], BF16, kind="ExternalInput")
    s2w = nc.dram_tensor("s2w", [DIM_A, TPC], BF16, kind="ExternalInput")
    ww = nc.dram_tensor("ww", [U, TPC * EMB], BF16, kind="ExternalInput")
    lne_out = nc.dram_tensor("lne", [P, TPC * EMB], BF16, kind="ExternalOutput")

    with tile.TileContext(nc) as tc:
        with (
            tc.tile_pool(name="const", bufs=1) as cpool,
            tc.tile_pool(name="gat", bufs=2) as gpool,
            tc.tile_pool(name="ohp", bufs=ohbufs) as ohpool,
            tc.tile_pool(name="work", bufs=6) as wpool,
            tc.tile_pool(name="ps_nte", bufs=2, space="PSUM") as ps_nte,
            tc.tile_pool(name="ps_tp", bufs=2, space="PSUM") as ps_tp,
            tc.tile_pool(name="ps_h", bufs=1, space="PSUM") as ps_h,
            tc.tile_pool(name="ps_lg", bufs=1, space="PSUM") as ps_lg,
            tc.tile_pool(name="ps_at", bufs=1, space="PSUM") as ps_at,
            tc.tile_pool(name="ps_dl", bufs=1, space="PSUM") as ps_dl,
        ):
            ident = cpool.tile([P, P], F32)
            make_identity(nc, ident[:])
            identb = cpool.tile([P, P], BF16)
            nc.vector.tensor_copy(out=identb[:], in_=ident[:])
            ttab_sb = cpool.tile([P, T, NBLK, U], BF16)
            nc.sync.dma_start(
                out=ttab_sb[:].rearrange("p t s u -> p (t s u)"), in_=ttab[:]
            )
            s1_sb = cpool.tile([U, TPC * DIM_A], BF16)
            nc.sync.dma_start(out=s1_sb[:], in_=s1w[:])
            s2_sb = cpool.tile([DIM_A, TPC], BF16)
            nc.sync.dma_start(out=s2_sb[:], in_=s2w[:])
            w_sb = cpool.tile([U, TPC * EMB], BF16)
            nc.sync.dma_start(out=w_sb[:], in_=ww[:])
            ne_idx_sb = cpool.tile([P, TPC * 8], I16)
            nc.sync.dma_start(out=ne_idx_sb[:], in_=neidx[:])
            with tc.For_i(0, reps, 1) if reps > 1 else contextlib.nullcontext():
                lne_sb = gpool.tile([P, TPC, EMB], BF16, tag="lne_sb")
                sscol = gpool.tile([P, TPC], F32, tag="sscol")
                # full-core node_embeddings gather: [128, TPC, 256]
                ne_g = gpool.tile([P, TPC, EMB], F32, tag="ne_g")
                ne_splits = np.linspace(0, TPC, NQ + 1).astype(int)
                for q in range(NQ):
                    lo, hi = int(ne_splits[q]), int(ne_splits[q + 1])
                    if hi == lo:
                        continue
                    nc.gpsimd.dma_gather(
                        ne_g[:, lo:hi, :],
                        ntab[:],
                        ne_idx_sb[:, lo * 8 : hi * 8],
                        (hi - lo) * P,
                        (hi - lo) * P,
                        EMB,
                        single_packet=False,
                        queue_num=q,
                    )
                # Software-pipelined tile loop: stage k of tile j runs in
                # iteration j+k, so every cross-engine dependency has a full
                # iteration (~3us) to resolve and the PE never stalls.
                nte_ps_l, nte_l, pt_l, ntet_l = {}, {}, {}, {}
                h_ps_l, h_l, lg_ps_l, att_l, at_ps_l = {}, {}, {}, {}, {}
                for i in range(TPC + 5):
                    if i < TPC:
                        j = i
                        ohj = ohpool.tile([P, T, NBLK, P], FP8, tag="ohj")
                        ring = nc.sync if j % 2 == 0 else nc.scalar
                        ring.dma_start(
                            out=ohj[:].rearrange("p t s b -> p (t s b)"),
                            in_=oh[:, j * OHC : (j + 1) * OHC],
                        )
                        nte_ps = ps_nte.tile([P, T * U], F32, tag="nte_ps")
                        for t in range(T if mode != "dma" else 0):
                            for s in range(NBLK):
                                nc.tensor.matmul(
                                    out=nte_ps[:, t * U : (t + 1) * U],
                                    lhsT=ohj[:, t, s, :],
                                    rhs=ttab_sb[:, t, s, :],
                                    start=(s == 0),
                                    stop=(s == NBLK - 1),
                                )
                        nte_ps_l[j] = nte_ps
                    if mode != "full":
                        nte_ps_l.clear()
                        continue
                    if 0 <= i - 1 < TPC:
                        j = i - 1
                        nte = wpool.tile([P, T * U], BF16, tag="nte")
                        nc.vector.tensor_copy(out=nte[:], in_=nte_ps_l.pop(j)[:])
                        nte_l[j] = nte
                        pt = ps_tp.tile([U, T * P], BF16, tag="pt")
                        for t in range(T):
                            nc.tensor.transpose(
                                pt[:, t * P : (t + 1) * P],
                                nte[:, t * U : (t + 1) * U],
                                identb[:],
                            )
                        pt_l[j] = pt
                    if 0 <= i - 2 < TPC:
                        j = i - 2
                        ntet = wpool.tile([U, T * P], BF16, tag="ntet")
                        nc.vector.tensor_copy(out=ntet[:], in_=pt_l.pop(j)[:])
                        ntet_l[j] = ntet
                        h_ps = ps_h.tile([DIM_A, T * P], F32, tag="h_ps")
                        nc.tensor.matmul(
                            out=h_ps[:],
                            lhsT=s1_sb[:, j * DIM_A : (j + 1) * DIM_A],
                            rhs=ntet[:],
                            start=True,
                            stop=True,
                        )
                        h = wpool.tile([DIM_A, T * P], BF16, tag="h")
                        nc.scalar.activation(h[:], h_ps[:], AF.Tanh)
                        h_l[j] = h
                    if 0 <= i - 3 < TPC:
                        j = i - 3
                        h = h_l.pop(j)
                        lg_ps = ps_lg.tile([P, T], F32, tag="lg_ps")
                        for t in range(T):
                            nc.tensor.matmul(
                                out=lg_ps[:, t : t + 1],
                                lhsT=h[:, t * P : (t + 1) * P],
                                rhs=s2_sb[:, j : j + 1],
                                start=True,
                                stop=True,
                            )
                        lg = wpool.tile([P, T], F32, tag="lg")
                        nc.vector.tensor_copy(out=lg[:], in_=lg_ps[:])
                        lg_ps_l[j] = lg
                    if 0 <= i - 4 < TPC:
                        j = i - 4
                        lg = lg_ps_l.pop(j)
                        nmax = wpool.tile([P, 1], F32, tag="nmax")
                        nc.vector.tensor_reduce(
                            out=nmax[:],
                            in_=lg[:],
                            axis=mybir.AxisListType.X,
                            op=ALU.max,
                            negate=True,
                        )
                        ex = wpool.tile([P, T], F32, tag="ex")
                        nc.scalar.activation(
                            ex[:], lg[:], AF.Exp, bias=nmax[:, 0:1]
                        )
                        ssum = wpool.tile([P, 1], F32, tag="ssum")
                        nc.vector.tensor_reduce(
                            out=ssum[:], in_=ex[:], axis=mybir.AxisListType.X,
                            op=ALU.add,
                        )
                        rs = wpool.tile([P, 1], F32, tag="rs")
                        nc.vector.reciprocal(rs[:], ssum[:])
                        att = wpool.tile([P, T], F32, tag="att")
                        nc.vector.tensor_scalar_mul(att[:], ex[:], rs[:, 0:1])
                        # agg[b,u] = sum_t att[b,t] * nte[b,t,u]
                        ntev = nte_l[j][:].rearrange("p (t u) -> p t u", t=T)
                        attb = att[:, :, None].to_broadcast([P, T, U])
                        tmp = wpool.tile([P, T, U], F32, tag="tmp")
                        nc.vector.tensor_tensor(
                            out=tmp[:], in0=ntev, in1=attb, op=ALU.mult
                        )
                        agg = wpool.tile([P, U], F32, tag="agg")
                        tmpv = tmp[:].rearrange("p t u -> p u t")
                        nc.vector.tensor_reduce(
                            out=agg[:], in_=tmpv, axis=mybir.AxisListType.X,
                            op=ALU.add,
                        )
                        nte_l.pop(j)
                        at_ps = ps_at.tile([U, P], F32, tag="at_ps")
                        nc.tensor.transpose(at_ps[:], agg[:], ident[:])
                        aggT = wpool.tile([U, P], BF16, tag="aggT")
                        nc.vector.tensor_copy(out=aggT[:], in_=at_ps[:])
                        at_ps_l[j] = aggT
                    if 0 <= i - 5 < TPC:
                        j = i - 5
                        aggT = at_ps_l.pop(j)
                        dl_ps = ps_dl.tile([P, EMB], F32, tag="dl_ps")
                        nc.tensor.matmul(
                            out=dl_ps[:],
                            lhsT=aggT[:],
                            rhs=w_sb[:, j * EMB : (j + 1) * EMB],
                            start=True,
                            stop=True,
                        )
                        nen = wpool.tile([P, EMB], F32, tag="nen")
                        nc.vector.tensor_add(
                            out=nen[:], in0=ne_g[:, j, :], in1=dl_ps[:]
                        )
                        sq = wpool.tile([P, EMB], F32, tag="sq")
                        nc.scalar.activation(
                            sq[:], nen[:], AF.Square, accum_out=sscol[:, j : j + 1]
                        )
                        nc.vector.tensor_copy(out=lne_sb[:, j, :], in_=nen[:])

                # ---- deferred row L2 norm: one Sqrt pass for all tiles
                if mode != "full":
                    nc.vector.memset(lne_sb[:], 0.0)
                nrm = wpool.tile([P, TPC], F32, tag="nrmcol")
                nc.scalar.activation(nrm[:], sscol[:], AF.Sqrt)
                nc.vector.tensor_scalar_max(nrm[:], nrm[:], 1e-12)
                inv = wpool.tile([P, TPC], F32, tag="invcol")
                nc.vector.reciprocal(inv[:], nrm[:])
                invb = inv[:, :, None].to_broadcast([P, TPC, EMB])
                nc.vector.tensor_tensor(
                    out=lne_sb[:], in0=lne_sb[:], in1=invb, op=ALU.mult
                )
                nc.sync.dma_start(
                    out=lne_out[:], in_=lne_sb[:].rearrange("p j e -> p (j e)")
                )

    nc.compile()
    _split_drain_waits(nc)
    return nc


# ---------------------------------------------------------------------------
# Phase-2 program builder: dense MT matmul segment-sum
# ---------------------------------------------------------------------------
def build_phase2(NB, reps=1):
    """NB: number of 128-row blocks in the lne table (== total sample tiles)."""
    nc = bacc.Bacc("TRN2", debug=False)
    lne = nc.dram_tensor("lne", [P, NB * EMB], BF16, kind="ExternalInput")
    mt = nc.dram_tensor("mt", [P, NB * 2 * P], FP8, kind="ExternalInput")
    recip = nc.dram_tensor("recip", [P, 2], F32, kind="ExternalInput")
    fcwt = nc.dram_tensor("fcwt", [EMB, EMBED_SIZE], BF16, kind="ExternalInput")
    fcb = nc.dram_tensor("fcb", [P, EMBED_SIZE], F32, kind="ExternalInput")
    out = nc.dram_tensor("out", [SEG_PER_CORE, EMBED_SIZE], F32, kind="ExternalOutput")

    with tile.TileContext(nc) as tc:
        with (
            tc.tile_pool(name="const", bufs=1) as cpool,
            tc.tile_pool(name="gat", bufs=4) as gpool,
            tc.tile_pool(name="work", bufs=6) as wpool,
            tc.tile_pool(name="ps_ac", bufs=1, space="PSUM") as ps_acc,
            tc.tile_pool(name="ps_tp", bufs=2, space="PSUM") as ps_tp,
            tc.tile_pool(name="ps_fc", bufs=2, space="PSUM") as ps_fc,
        ):
            ident = cpool.tile([P, P], F32)
            make_identity(nc, ident[:])
            recip_sb = cpool.tile([P, 2], F32)
            nc.sync.dma_start(out=recip_sb[:], in_=recip[:])
            fcwt0 = cpool.tile([P, EMBED_SIZE], BF16, tag="fcwt0")
            fcwt1 = cpool.tile([P, EMBED_SIZE], BF16, tag="fcwt1")
            fcwt_sb = [fcwt0, fcwt1]
            for i in range(2):
                nc.sync.dma_start(out=fcwt_sb[i][:], in_=fcwt[i * P : (i + 1) * P, :])
            fcb_sb = cpool.tile([P, EMBED_SIZE], F32)
            nc.sync.dma_start(out=fcb_sb[:], in_=fcb[:])
            eps8 = cpool.tile([P, 1], F32)
            nc.vector.memset(eps8[:], 1e-8)
            # preload the Sqrt act table before the matmul stream
            warm = cpool.tile([P, 1], F32)
            nc.scalar.activation(warm[:], eps8[:], AF.Sqrt)

            with tc.For_i(0, reps, 1) if reps > 1 else contextlib.nullcontext():
                acc0 = ps_acc.tile([P, EMB], F32, tag="acc0")
                acc1 = ps_acc.tile([P, EMB], F32, tag="acc1")
                acc = [acc0, acc1]
                done = 0
                ci = 0
                while done < NB:
                    nt = min(P2_CHUNK, NB - done)
                    ring = nc.sync if ci % 2 == 0 else nc.scalar
                    mtc = gpool.tile([P, P2_CHUNK, 2, P], FP8, tag="mtc")
                    ring.dma_start(
                        out=mtc[:, :nt, :, :].rearrange("p k m b -> p (k m b)"),
                        in_=mt[:, done * 2 * P : (done + nt) * 2 * P],
                    )
                    lnec = gpool.tile([P, P2_CHUNK, EMB], BF16, tag="lnec")
                    ring.dma_start(
                        out=lnec[:, :nt, :].rearrange("p k e -> p (k e)"),
                        in_=lne[:, done * EMB : (done + nt) * EMB],
                    )
                    for kk in range(nt):
                        k = done + kk
                        for m in range(2):
                            nc.tensor.matmul(
                                out=acc[m][:],
                                lhsT=mtc[:, kk, m, :],
                                rhs=lnec[:, kk, :],
                                start=(k == 0),
                                stop=(k == NB - 1),
                            )
                    done += nt
                    ci += 1
                # ---- mean + normalize (batched over both 128-seg halves)
                meanb = wpool.tile([P, 2, EMB], F32, tag="meanb")
                for half in range(2):
                    nc.vector.tensor_scalar_mul(
                        meanb[:, half, :], acc[half][:], recip_sb[:, half : half + 1]
                    )
                sqm = wpool.tile([P, 2, EMB], F32, tag="sqm")
                nc.vector.tensor_tensor(
                    out=sqm[:], in0=meanb[:], in1=meanb[:], op=ALU.mult
                )
                ss2 = wpool.tile([P, 2], F32, tag="ss2")
                nc.vector.tensor_reduce(
                    out=ss2[:], in_=sqm[:], axis=mybir.AxisListType.X, op=ALU.add
                )
                nrm2 = wpool.tile([P, 2], F32, tag="nrm2")
                nc.scalar.activation(nrm2[:], ss2[:], AF.Sqrt)
                nc.vector.tensor_scalar_max(nrm2[:], nrm2[:], 1e-12)
                inv2 = wpool.tile([P, 2], F32, tag="inv2")
                nc.vector.reciprocal(inv2[:], nrm2[:])
                smn = wpool.tile([P, 2, EMB], F32, tag="smn")
                inv2b = inv2[:, :, None].to_broadcast([P, 2, EMB])
                nc.vector.tensor_tensor(
                    out=smn[:], in0=meanb[:], in1=inv2b, op=ALU.mult
                )
                # ---- transpose smn -> smnT (bf16) [emb-half][128, 256(seg)]
                smnT = []
                for eh in range(2):
                    tp = ps_tp.tile([P, 2 * P], F32, tag="tp")
                    for half in range(2):
                        nc.tensor.transpose(
                            tp[:, half * P : (half + 1) * P],
                            smn[:, half, eh * P : (eh + 1) * P],
                            ident[:],
                        )
                    st = wpool.tile([P, 2 * P], BF16, tag=f"smnT{eh}")
                    nc.vector.tensor_copy(out=st[:], in_=tp[:])
                    smnT.append(st)
                # ---- FC + bias + l2norm (batched over both seg halves)
                xx = wpool.tile([P, 2, EMBED_SIZE], F32, tag="xx")
                for m in range(2):
                    fc_ps = ps_fc.tile([P, EMBED_SIZE], F32, tag="fc_ps")
                    for kh in range(2):
                        nc.tensor.matmul(
                            out=fc_ps[:],
                            lhsT=smnT[kh][:, m * P : (m + 1) * P],
                            rhs=fcwt_sb[kh][:],
                            start=(kh == 0),
                            stop=(kh == 1),
                        )
                    nc.vector.tensor_add(out=xx[:, m, :], in0=fc_ps[:], in1=fcb_sb[:])
                sq3 = wpool.tile([P, 2, EMBED_SIZE], F32, tag="sq3")
                nc.vector.tensor_tensor(out=sq3[:], in0=xx[:], in1=xx[:], op=ALU.mult)
                ss3 = wpool.tile([P, 2], F32, tag="ss3")
                nc.vector.tensor_reduce(
                    out=ss3[:], in_=sq3[:], axis=mybir.AxisListType.X, op=ALU.add
                )
                nrm3 = wpool.tile([P, 2], F32, tag="nrm3")
                # n = sqrt(ss + 1e-8) + 1e-8
                nc.scalar.activation(nrm3[:], ss3[:], AF.Sqrt, bias=eps8[:, 0:1])
                nc.vector.tensor_scalar_add(nrm3[:], nrm3[:], 1e-8)
                inv3 = wpool.tile([P, 2], F32, tag="inv3")
                nc.vector.reciprocal(inv3[:], nrm3[:])
                res = wpool.tile([P, 2, EMBED_SIZE], F32, tag="res")
                inv3b = inv3[:, :, None].to_broadcast([P, 2, EMBED_SIZE])
                nc.vector.tensor_tensor(out=res[:], in0=xx[:], in1=inv3b, op=ALU.mult)
                for m in range(2):
                    nc.sync.dma_start(
                        out=out[m * P : (m + 1) * P, :], in_=res[:, m, :]
                    )

    nc.compile()
    _split_drain_waits(nc)
    return nc


# ---------------------------------------------------------------------------
# Host-side orchestration
# ---------------------------------------------------------------------------
def _phase1_prep(train_inputs, train_types, node_neigh):
    order = np.argsort(train_types, kind="stable")
    ts = train_types[order]
    tiles_s, tiles_t = [], []
    for t in range(T):
        idx_t = order[ts == t]
        if len(idx_t) == 0:
            continue
        n_tiles = -(-len(idx_t) // P)
        padded = np.concatenate(
            [idx_t, np.repeat(idx_t[-1:], n_tiles * P - len(idx_t))]
        )
        for jj in range(n_tiles):
            tiles_s.append(padded[jj * P : (jj + 1) * P])
            tiles_t.append(t)
    while len(tiles_s) % N_CORES:
        tiles_s.append(tiles_s[-1])
        tiles_t.append(tiles_t[-1])
    sample_mat = np.stack(tiles_s)  # [TT, 128]
    tile_type = np.asarray(tiles_t)
    TT = sample_mat.shape[0]
    TPC = TT // N_CORES

    flat = sample_mat.reshape(-1)
    slot_of_sample = np.zeros(B, np.int64)
    slot_of_sample[flat[::-1]] = np.arange(TT * P)[::-1]
    return sample_mat, tile_type, TPC, slot_of_sample


def _phase1_inmaps(inputs, sample_mat, tile_type, TPC):
    node_embeddings = np.asarray(inputs["node_embeddings"], np.float32)
    node_type_embeddings = np.asarray(inputs["node_type_embeddings"], np.float32)
    trans_weights = np.asarray(inputs["trans_weights"], np.float32)
    trans_weights_s1 = np.asarray(inputs["trans_weights_s1"], np.float32)
    trans_weights_s2 = np.asarray(inputs["trans_weights_s2"], np.float32)
    train_inputs = np.asarray(inputs["train_inputs"])
    node_neigh = np.asarray(inputs["node_neigh"])

    # ttab in node-block layout: [p, (t, s, u)] = ntype[s*128+p, t, u]
    npad = NBLK * P  # 2560
    ttab_pad = np.zeros((npad, T, U), np.float32)
    ttab_pad[:NUM_NODES] = node_type_embeddings
    ttab_blk = np.ascontiguousarray(
        ttab_pad.reshape(NBLK, P, T, U).transpose(1, 2, 0, 3).reshape(P, T * NBLK * U)
    ).astype(NPBF16)

    OHC = T * NBLK * P
    in_maps = []
    for k in range(N_CORES):
        smp = sample_mat[k * TPC : (k + 1) * TPC]  # [TPC, 128]
        ct = tile_type[k * TPC : (k + 1) * TPC]  # [TPC]
        ne_flat = train_inputs[smp].reshape(-1)  # order: tile-major, then p
        ne_idx = _wrap16(ne_flat)
        # one-hot neighbor counts: oh[p, ((j*T+t)*NBLK+s)*128+b]
        nn = node_neigh[smp]  # [TPC, 128, T, 10]
        s_i = nn // P
        p_i = nn % P
        jj = np.arange(TPC)[:, None, None, None]
        bb = np.arange(P)[None, :, None, None]
        tt = np.arange(T)[None, None, :, None]
        col = ((jj * T + tt) * NBLK + s_i) * P + bb
        oh_k = np.zeros((P, TPC * OHC), np.uint8)
        np.add.at(oh_k, (p_i.ravel(), col.ravel()), 1)
        s1_all = np.ascontiguousarray(
            trans_weights_s1[ct].transpose(1, 0, 2).reshape(U, TPC * DIM_A)
        ).astype(NPBF16)
        w_all = np.ascontiguousarray(
            trans_weights[ct].transpose(1, 0, 2).reshape(U, TPC * EMB)
        ).astype(NPBF16)
        s2_blk = np.ascontiguousarray(trans_weights_s2[ct][:, :, 0].T).astype(
            NPBF16
        )  # [32, TPC]
        in_maps.append(
            {
                "ttab": ttab_blk,
                "oh": oh_k.astype(NPFP8),
                "ntab": node_embeddings,
                "neidx": ne_idx,
                "s1w": s1_all,
                "s2w": s2_blk,
                "ww": w_all,
            }
        )
    return in_maps


def _phase2_prep(region_index, region_segment_ids, slot_of_sample, NB):
    """Per-core fp8 count matrices MT[p, (k*2+m)*128+seg_l] for the dense
    segment-sum matmul, plus per-segment reciprocal counts."""
    seg_ids = np.asarray(region_segment_ids).astype(np.int64)
    new_idx = slot_of_sample[np.asarray(region_index).astype(np.int64)]
    k_i = new_idx // P
    p_i = new_idx % P
    core = seg_ids // SEG_PER_CORE
    m_i = (seg_ids % SEG_PER_CORE) // P
    sl = seg_ids % P
    col = (k_i * 2 + m_i) * P + sl
    mt = np.zeros((N_CORES, P, NB * 2 * P), np.uint8)
    np.add.at(mt, (core, p_i, col), 1)
    assert mt.max() <= 16, "fp8e4 exact-integer range exceeded"

    cnt = np.bincount(seg_ids, minlength=NSEG).astype(np.float32)
    recip_all = np.where(cnt > 0, 1.0 / np.maximum(cnt, 1.0), 0.0).astype(np.float32)

    mt_l, recip_l = [], []
    for k in range(N_CORES):
        mt_l.append(mt[k].astype(NPFP8))
        rc = recip_all[k * SEG_PER_CORE : (k + 1) * SEG_PER_CORE]
        recip_l.append(np.ascontiguousarray(rc.reshape(2, P).T))
    return mt_l, recip_l


def _phase2_inmaps(inputs, lne_cat, mt_l, recip_l):
    fc_w = np.asarray(inputs["fc_w"], np.float32)
    fc_b = np.asarray(inputs["fc_b"], np.float32)
    fcwt = np.ascontiguousarray(fc_w.T).astype(NPBF16)  # [256, 512]
    fcb = np.broadcast_to(fc_b[None, :], (P, EMBED_SIZE)).copy()
    in_maps = []
    for k in range(N_CORES):
        in_maps.append(
            {
                "lne": lne_cat,
                "mt": mt_l[k],
                "recip": recip_l[k],
                "fcwt": fcwt,
                "fcb": fcb,
            }
        )
    return in_maps


def _run_spmd_retry(nc, in_maps, retries=3, delay=45.0):
    """The axon-tunneled device occasionally reports a transient
    UNAVAILABLE/unrecoverable state; back off and retry."""
    import time as _time

    last = None
    for attempt in range(retries):
        try:
            return run_bass_kernel_spmd(nc, in_maps, list(range(N_CORES)))
        except Exception as e:  # jax.errors.JaxRuntimeError and friends
            last = e
            if attempt + 1 < retries:
                _time.sleep(delay)
    raise last


_P1_CACHE = {}
_P2_CACHE = {}


def kernel(**inputs) -> np.ndarray:
    train_inputs = np.asarray(inputs["train_inputs"])
    train_types = np.asarray(inputs["train_types"])
    node_neigh = np.asarray(inputs["node_neigh"])
    num_sms = int(inputs["num_sms"])
    max_region = int(inputs["max_region"])

    sample_mat, tile_type, TPC, slot_of_sample = _phase1_prep(
        train_inputs, train_types, node_neigh
    )
    TT = sample_mat.shape[0]

    if TPC not in _P1_CACHE:
        _P1_CACHE[TPC] = build_phase1(TPC)
    nc1 = _P1_CACHE[TPC]
    in_maps1 = _phase1_inmaps(inputs, sample_mat, tile_type, TPC)
    res1 = _run_spmd_retry(nc1, in_maps1).results

    # relay: pure concat — lne row (tile k*TPC+j, p) lives at [p, tile*256+e]
    lne_cat = np.concatenate([res1[k]["lne"] for k in range(N_CORES)], axis=1)

    NB = TT
    mt_l, recip_l = _phase2_prep(
        inputs["region_index"], inputs["region_segment_ids"], slot_of_sample, NB
    )
    if NB not in _P2_CACHE:
        _P2_CACHE[NB] = build_phase2(NB)
    nc2 = _P2_CACHE[NB]
    in_maps2 = _phase2_inmaps(inputs, lne_cat, mt_l, recip_l)
    res2 = _run_spmd_retry(nc2, in_maps2).results

    out = np.concatenate([res2[k]["out"] for k in range(N_CORES)], axis=0)
    return out.reshape(num_sms, max_region, EMBED_SIZE)


# revision 22
# speedup vs baseline: 1.1548x; 1.1548x over previous
"""GATNE model on 8 Trainium2 NeuronCores via Bass/Tile — v3.

v3 replaces both descriptor-limited dma_gathers with dense-DMA + PE matmuls
(measured: dma_gather ~2.4-3.9 ns/row vs dense DMA ~360+ GB/s; PE matmul
supports MIXED operand dtypes — fp8e4 count matrices x bf16 tables — at
~0.47 ns/row with N=64, verified exact for integer counts <= 16):

Phase 1 (SPMD, data-parallel over B, samples sorted by type so tiles are
type-homogeneous): the neighbor gather-sum becomes, per (tile, type), a
K=2560-node one-hot COUNT matmul: nte[b,u] = sum_s OH[s-block,b]^T @
ttab_t[s-block,u], counts shipped from host as fp8e4 (exact: counts<=10),
ttab resident in SBUF as bf16. Attention/delta/normalize as before.
Phase 2 (SPMD, sharded by 256-segment blocks): segment-sum becomes a dense
matmul: acc[seg,e] = sum_k MT[k-block,seg]^T @ lne[k-block,e] with MT the
fp8e4 per-(sample-slot, segment) count matrix and lne the bf16 embedding
table streamed block-wise. Mean+normalize, FC, l2norm as before.
"""
import contextlib
import sys

sys.path.insert(0, "/opt/trn_rl_repo")

import numpy as np
import ml_dtypes

import concourse.bacc as bacc
import concourse.bass as bass
import concourse.mybir as mybir
import concourse.tile as tile
from concourse.bass_utils import run_bass_kernel_spmd
from concourse.masks import make_identity

F32 = mybir.dt.float32
BF16 = mybir.dt.bfloat16
FP8 = mybir.dt.float8e4
I16 = mybir.dt.int16
AF = mybir.ActivationFunctionType
ALU = mybir.AluOpType
NPBF16 = ml_dtypes.bfloat16
NPFP8 = ml_dtypes.float8_e4m3

N_CORES = 8
NUM_NODES = 2500
T = 4
EMB = 256
U = 64
DIM_A = 32
EMBED_SIZE = 512
NEIGH = 10
B = NUM_NODES * T
NUM_SMS = 64
MAX_REGION = 32
NSEG = NUM_SMS * MAX_REGION  # 2048
SEG_PER_CORE = NSEG // N_CORES  # 256
E_TOTAL = 131072
P = 128

NQ = 4  # SWDGE queues (node_embeddings gather only)
NBLK = 20  # 2560-padded node blocks of 128
P2_CHUNK = 10  # K-blocks per phase-2 stream chunk


def _wrap16(flat):
    """dma_gather index layout: idx i -> partition i%16, column i//16,
    replicated across the 8 partition groups (one per Q7 core)."""
    n = flat.shape[0]
    assert n % 16 == 0
    a = flat.reshape(n // 16, 16).T.astype(np.int16)  # [16, n//16]
    return np.tile(a, (8, 1))  # [128, n//16]


# ---------------------------------------------------------------------------
# walrus post-pass: CoreV3 codegen rejects >1 sem wait on a TPB_CTRL (Drain);
# split the excess onto injected wait-only drains placed just before.
# ---------------------------------------------------------------------------
def _split_drain_waits(nc, max_waits=1):
    for bb in nc.main_func.blocks:
        out = []
        for ins in bb.instructions:
            si = ins.sync_info
            if (
                type(ins).__name__ == "InstDrain"
                and si is not None
                and si.on_wait is not None
                and len(si.on_wait) > max_waits
            ):
                waits = list(si.on_wait)
                extra, keep = waits[:-max_waits], waits[-max_waits:]
                for i in range(0, len(extra), max_waits):
                    d = mybir.InstDrain(
                        name=nc.get_next_instruction_name(),
                        ins=[],
                        outs=[],
                        bass_is_fusable=False,
                    )
                    d.engine = ins.engine
                    d.sync_info = mybir.SyncInfo(
                        on_wait=extra[i : i + max_waits], on_update=[]
                    )
                    out.append(d)
                ins.sync_info = mybir.SyncInfo(
                    on_wait=keep, on_update=list(si.on_update or [])
                )
            out.append(ins)
        bb.instructions = out
    return nc


# ---------------------------------------------------------------------------
# Phase-1 program builder
# ---------------------------------------------------------------------------
def build_phase1(TPC, reps=1, ohbufs=4, mode='full'):
    OHC = T * NBLK * P  # one-hot columns per tile (4*20*128)
    nc = bacc.Bacc("TRN2", debug=False, num_swdge_queues=NQ)
    ttab = nc.dram_tensor("ttab", [P, T * NBLK * U], BF16, kind="ExternalInput")
    oh = nc.dram_tensor("oh", [P, TPC * OHC], FP8, kind="ExternalInput")
    ntab = nc.dram_tensor("ntab", [NUM_NODES, EMB], BF16, kind="ExternalInput")
    neidx = nc.dram_tensor("neidx", [P, TPC * 8], I16, kind="ExternalInput")
    s1w = nc.dram_tensor("s1w", [U, TPC * DIM_A], BF16, kind="ExternalInput")
    s2w = nc.dram_tensor("s2w", [DIM_A, TPC], BF16, kind="ExternalInput")
    ww = nc.dram_tensor("ww", [U, TPC * EMB], BF16, kind="ExternalInput")
    lne_out = nc.dram_tensor("lne", [P, TPC * EMB], BF16, kind="ExternalOutput")

    with tile.TileContext(nc) as tc:
        with (
            tc.tile_pool(name="const", bufs=1) as cpool,
            tc.tile_pool(name="gat", bufs=2) as gpool,
            tc.tile_pool(name="ohp", bufs=ohbufs) as ohpool,
            tc.tile_pool(name="work", bufs=6) as wpool,
            tc.tile_pool(name="ps_nte", bufs=2, space="PSUM") as ps_nte,
            tc.tile_pool(name="ps_tp", bufs=2, space="PSUM") as ps_tp,
            tc.tile_pool(name="ps_h", bufs=1, space="PSUM") as ps_h,
            tc.tile_pool(name="ps_lg", bufs=1, space="PSUM") as ps_lg,
            tc.tile_pool(name="ps_at", bufs=1, space="PSUM") as ps_at,
            tc.tile_pool(name="ps_dl", bufs=1, space="PSUM") as ps_dl,
        ):
            ident = cpool.tile([P, P], F32)
            make_identity(nc, ident[:])
            identb = cpool.tile([P, P], BF16)
            nc.vector.tensor_copy(out=identb[:], in_=ident[:])
            ttab_sb = cpool.tile([P, T, NBLK, U], BF16)
            nc.sync.dma_start(
                out=ttab_sb[:].rearrange("p t s u -> p (t s u)"), in_=ttab[:]
            )
            s1_sb = cpool.tile([U, TPC * DIM_A], BF16)
            nc.sync.dma_start(out=s1_sb[:], in_=s1w[:])
            s2_sb = cpool.tile([DIM_A, TPC], BF16)
            nc.sync.dma_start(out=s2_sb[:], in_=s2w[:])
            w_sb = cpool.tile([U, TPC * EMB], BF16)
            nc.sync.dma_start(out=w_sb[:], in_=ww[:])
            ne_idx_sb = cpool.tile([P, TPC * 8], I16)
            nc.sync.dma_start(out=ne_idx_sb[:], in_=neidx[:])
            with tc.For_i(0, reps, 1) if reps > 1 else contextlib.nullcontext():
                lne_sb = gpool.tile([P, TPC, EMB], BF16, tag="lne_sb")
                sscol = gpool.tile([P, TPC], F32, tag="sscol")
                # full-core node_embeddings gather: [128, TPC, 256]
                ne_g = gpool.tile([P, TPC, EMB], BF16, tag="ne_g")
                ne_splits = np.linspace(0, TPC, NQ + 1).astype(int)
                # Software-pipelined tile loop: stage k of tile j runs in
                # iteration j+k, so every cross-engine dependency has a full
                # iteration (~3us) to resolve and the PE never stalls.
                nte_ps_l, nte_l, pt_l, ntet_l = {}, {}, {}, {}
                h_ps_l, h_l, lg_ps_l, att_l, at_ps_l = {}, {}, {}, {}, {}
                for i in range(TPC + 5):
                    if i == 2:
                        for q in range(NQ):
                            lo, hi = int(ne_splits[q]), int(ne_splits[q + 1])
                            if hi == lo:
                                continue
                            nc.gpsimd.dma_gather(
                                ne_g[:, lo:hi, :],
                                ntab[:],
                                ne_idx_sb[:, lo * 8 : hi * 8],
                                (hi - lo) * P,
                                (hi - lo) * P,
                                EMB,
                                single_packet=False,
                                queue_num=q,
                            )
                    if i < TPC:
                        j = i
                        ohj = ohpool.tile([P, T, NBLK, P], FP8, tag="ohj")
                        ring = nc.sync if j % 2 == 0 else nc.gpsimd
                        ring.dma_start(
                            out=ohj[:].rearrange("p t s b -> p (t s b)"),
                            in_=oh[:, j * OHC : (j + 1) * OHC],
                        )
                        if mode == "dma":
                            nc.vector.tensor_copy(
                                out=lne_sb[:, 0, 0:P], in_=ohj[:, 0, 0, :]
                            )
                            continue
                        nte_ps = ps_nte.tile([P, T * U], F32, tag="nte_ps")
                        for t in range(T):
                            for s in range(NBLK):
                                nc.tensor.matmul(
                                    out=nte_ps[:, t * U : (t + 1) * U],
                                    lhsT=ohj[:, t, s, :],
                                    rhs=ttab_sb[:, t, s, :],
                                    start=(s == 0),
                                    stop=(s == NBLK - 1),
                                )
                        nte_ps_l[j] = nte_ps
                    if mode != "full":
                        nte_ps_l.clear()
                        continue
                    if 0 <= i - 1 < TPC:
                        j = i - 1
                        nte = wpool.tile([P, T * U], BF16, tag="nte")
                        nc.vector.tensor_copy(out=nte[:], in_=nte_ps_l.pop(j)[:])
                        nte_l[j] = nte
                        pt = ps_tp.tile([U, T * P], BF16, tag="pt")
                        for t in range(T):
                            nc.tensor.transpose(
                                pt[:, t * P : (t + 1) * P],
                                nte[:, t * U : (t + 1) * U],
                                identb[:],
                            )
                        pt_l[j] = pt
                    if 0 <= i - 2 < TPC:
                        j = i - 2
                        ntet = wpool.tile([U, T * P], BF16, tag="ntet")
                        nc.vector.tensor_copy(out=ntet[:], in_=pt_l.pop(j)[:])
                        ntet_l[j] = ntet
                        h_ps = ps_h.tile([DIM_A, T * P], F32, tag="h_ps")
                        nc.tensor.matmul(
                            out=h_ps[:],
                            lhsT=s1_sb[:, j * DIM_A : (j + 1) * DIM_A],
                            rhs=ntet[:],
                            start=True,
                            stop=True,
                        )
                        h = wpool.tile([DIM_A, T * P], BF16, tag="h")
                        nc.scalar.activation(h[:], h_ps[:], AF.Tanh)
                        h_l[j] = h
                    if 0 <= i - 3 < TPC:
                        j = i - 3
                        h = h_l.pop(j)
                        lg_ps = ps_lg.tile([P, T], F32, tag="lg_ps")
                        for t in range(T):
                            nc.tensor.matmul(
                                out=lg_ps[:, t : t + 1],
                                lhsT=h[:, t * P : (t + 1) * P],
                                rhs=s2_sb[:, j : j + 1],
                                start=True,
                                stop=True,
                            )
                        lg = wpool.tile([P, T], F32, tag="lg")
                        nc.vector.tensor_copy(out=lg[:], in_=lg_ps[:])
                        lg_ps_l[j] = lg
                    if 0 <= i - 4 < TPC:
                        j = i - 4
                        lg = lg_ps_l.pop(j)
                        nmax = wpool.tile([P, 1], F32, tag="nmax")
                        nc.vector.tensor_reduce(
                            out=nmax[:],
                            in_=lg[:],
                            axis=mybir.AxisListType.X,
                            op=ALU.max,
                            negate=True,
                        )
                        ex = wpool.tile([P, T], F32, tag="ex")
                        nc.scalar.activation(
                            ex[:], lg[:], AF.Exp, bias=nmax[:, 0:1]
                        )
                        ssum = wpool.tile([P, 1], F32, tag="ssum")
                        nc.vector.tensor_reduce(
                            out=ssum[:], in_=ex[:], axis=mybir.AxisListType.X,
                            op=ALU.add,
                        )
                        rs = wpool.tile([P, 1], F32, tag="rs")
                        nc.vector.reciprocal(rs[:], ssum[:])
                        att = wpool.tile([P, T], F32, tag="att")
                        nc.vector.tensor_scalar_mul(att[:], ex[:], rs[:, 0:1])
                        # agg[b,u] = sum_t att[b,t] * nte[b,t,u]
                        ntev = nte_l[j][:].rearrange("p (t u) -> p t u", t=T)
                        attb = att[:, :, None].to_broadcast([P, T, U])
                        tmp = wpool.tile([P, T, U], F32, tag="tmp")
                        nc.vector.tensor_tensor(
                            out=tmp[:], in0=ntev, in1=attb, op=ALU.mult
                        )
                        agg = wpool.tile([P, U], F32, tag="agg")
                        tmpv = tmp[:].rearrange("p t u -> p u t")
                        nc.vector.tensor_reduce(
                            out=agg[:], in_=tmpv, axis=mybir.AxisListType.X,
                            op=ALU.add,
                        )
                        nte_l.pop(j)
                        at_ps = ps_at.tile([U, P], F32, tag="at_ps")
                        nc.tensor.transpose(at_ps[:], agg[:], ident[:])
                        aggT = wpool.tile([U, P], BF16, tag="aggT")
                        nc.vector.tensor_copy(out=aggT[:], in_=at_ps[:])
                        at_ps_l[j] = aggT
                    if 0 <= i - 5 < TPC:
                        j = i - 5
                        aggT = at_ps_l.pop(j)
                        dl_ps = ps_dl.tile([P, EMB], F32, tag="dl_ps")
                        nc.tensor.matmul(
                            out=dl_ps[:],
                            lhsT=aggT[:],
                            rhs=w_sb[:, j * EMB : (j + 1) * EMB],
                            start=True,
                            stop=True,
                        )
                        nen = wpool.tile([P, EMB], F32, tag="nen")
                        nc.vector.tensor_add(
                            out=nen[:], in0=ne_g[:, j, :], in1=dl_ps[:]
                        )
                        sq = wpool.tile([P, EMB], F32, tag="sq")
                        nc.scalar.activation(
                            sq[:], nen[:], AF.Square, accum_out=sscol[:, j : j + 1]
                        )
                        nc.vector.tensor_copy(out=lne_sb[:, j, :], in_=nen[:])

                # ---- deferred row L2 norm: one Sqrt pass for all tiles
                if mode == "full":
                    nrm = wpool.tile([P, TPC], F32, tag="nrmcol")
                    nc.scalar.activation(nrm[:], sscol[:], AF.Sqrt)
                    nc.vector.tensor_scalar_max(nrm[:], nrm[:], 1e-12)
                    inv = wpool.tile([P, TPC], F32, tag="invcol")
                    nc.vector.reciprocal(inv[:], nrm[:])
                    invb = inv[:, :, None].to_broadcast([P, TPC, EMB])
                    nc.vector.tensor_tensor(
                        out=lne_sb[:], in0=lne_sb[:], in1=invb, op=ALU.mult
                    )
                else:
                    nc.vector.memset(lne_sb[:], 0.0)
                    nc.vector.memset(sscol[:], 0.0)
                nc.sync.dma_start(
                    out=lne_out[:], in_=lne_sb[:].rearrange("p j e -> p (j e)")
                )

    nc.compile()
    _split_drain_waits(nc)
    return nc


# ---------------------------------------------------------------------------
# Phase-2 program builder: dense MT matmul segment-sum
# ---------------------------------------------------------------------------
def build_phase2(NB, reps=1):
    """NB: number of 128-row blocks in the lne table (== total sample tiles)."""
    nc = bacc.Bacc("TRN2", debug=False)
    lne = nc.dram_tensor("lne", [P, NB * EMB], BF16, kind="ExternalInput")
    mt = nc.dram_tensor("mt", [P, NB * 2 * P], FP8, kind="ExternalInput")
    recip = nc.dram_tensor("recip", [P, 2], F32, kind="ExternalInput")
    fcwt = nc.dram_tensor("fcwt", [EMB, EMBED_SIZE], BF16, kind="ExternalInput")
    fcb = nc.dram_tensor("fcb", [P, EMBED_SIZE], F32, kind="ExternalInput")
    out = nc.dram_tensor("out", [SEG_PER_CORE, EMBED_SIZE], F32, kind="ExternalOutput")

    with tile.TileContext(nc) as tc:
        with (
            tc.tile_pool(name="const", bufs=1) as cpool,
            tc.tile_pool(name="gat", bufs=4) as gpool,
            tc.tile_pool(name="work", bufs=6) as wpool,
            tc.tile_pool(name="ps_ac", bufs=1, space="PSUM") as ps_acc,
            tc.tile_pool(name="ps_tp", bufs=2, space="PSUM") as ps_tp,
            tc.tile_pool(name="ps_fc", bufs=2, space="PSUM") as ps_fc,
        ):
            ident = cpool.tile([P, P], F32)
            make_identity(nc, ident[:])
            recip_sb = cpool.tile([P, 2], F32)
            nc.sync.dma_start(out=recip_sb[:], in_=recip[:])
            fcwt0 = cpool.tile([P, EMBED_SIZE], BF16, tag="fcwt0")
            fcwt1 = cpool.tile([P, EMBED_SIZE], BF16, tag="fcwt1")
            fcwt_sb = [fcwt0, fcwt1]
            for i in range(2):
                nc.sync.dma_start(out=fcwt_sb[i][:], in_=fcwt[i * P : (i + 1) * P, :])
            fcb_sb = cpool.tile([P, EMBED_SIZE], F32)
            nc.sync.dma_start(out=fcb_sb[:], in_=fcb[:])
            eps8 = cpool.tile([P, 1], F32)
            nc.vector.memset(eps8[:], 1e-8)
            # preload the Sqrt act table before the matmul stream
            warm = cpool.tile([P, 1], F32)
            nc.scalar.activation(warm[:], eps8[:], AF.Sqrt)

            with tc.For_i(0, reps, 1) if reps > 1 else contextlib.nullcontext():
                acc0 = ps_acc.tile([P, EMB], F32, tag="acc0")
                acc1 = ps_acc.tile([P, EMB], F32, tag="acc1")
                acc = [acc0, acc1]
                done = 0
                ci = 0
                while done < NB:
                    nt = min(P2_CHUNK, NB - done)
                    ring = nc.sync if ci % 2 == 0 else nc.scalar
                    mtc = gpool.tile([P, P2_CHUNK, 2, P], FP8, tag="mtc")
                    ring.dma_start(
                        out=mtc[:, :nt, :, :].rearrange("p k m b -> p (k m b)"),
                        in_=mt[:, done * 2 * P : (done + nt) * 2 * P],
                    )
                    lnec = gpool.tile([P, P2_CHUNK, EMB], BF16, tag="lnec")
                    ring.dma_start(
                        out=lnec[:, :nt, :].rearrange("p k e -> p (k e)"),
                        in_=lne[:, done * EMB : (done + nt) * EMB],
                    )
                    for kk in range(nt):
                        k = done + kk
                        for m in range(2):
                            nc.tensor.matmul(
                                out=acc[m][:],
                                lhsT=mtc[:, kk, m, :],
                                rhs=lnec[:, kk, :],
                                start=(k == 0),
                                stop=(k == NB - 1),
                            )
                    done += nt
                    ci += 1
                # ---- mean + normalize (batched over both 128-seg halves)
                meanb = wpool.tile([P, 2, EMB], F32, tag="meanb")
                for half in range(2):
                    nc.vector.tensor_scalar_mul(
                        meanb[:, half, :], acc[half][:], recip_sb[:, half : half + 1]
                    )
                sqm = wpool.tile([P, 2, EMB], F32, tag="sqm")
                nc.vector.tensor_tensor(
                    out=sqm[:], in0=meanb[:], in1=meanb[:], op=ALU.mult
                )
                ss2 = wpool.tile([P, 2], F32, tag="ss2")
                nc.vector.tensor_reduce(
                    out=ss2[:], in_=sqm[:], axis=mybir.AxisListType.X, op=ALU.add
                )
                nrm2 = wpool.tile([P, 2], F32, tag="nrm2")
                nc.scalar.activation(nrm2[:], ss2[:], AF.Sqrt)
                nc.vector.tensor_scalar_max(nrm2[:], nrm2[:], 1e-12)
                inv2 = wpool.tile([P, 2], F32, tag="inv2")
                nc.vector.reciprocal(inv2[:], nrm2[:])
                smn = wpool.tile([P, 2, EMB], F32, tag="smn")
                inv2b = inv2[:, :, None].to_broadcast([P, 2, EMB])
                nc.vector.tensor_tensor(
                    out=smn[:], in0=meanb[:], in1=inv2b, op=ALU.mult
                )
                # ---- transpose smn -> smnT (bf16) [emb-half][128, 256(seg)]
                smnT = []
                for eh in range(2):
                    tp = ps_tp.tile([P, 2 * P], F32, tag="tp")
                    for half in range(2):
                        nc.tensor.transpose(
                            tp[:, half * P : (half + 1) * P],
                            smn[:, half, eh * P : (eh + 1) * P],
                            ident[:],
                        )
                    st = wpool.tile([P, 2 * P], BF16, tag=f"smnT{eh}")
                    nc.vector.tensor_copy(out=st[:], in_=tp[:])
                    smnT.append(st)
                # ---- FC + bias + l2norm (batched over both seg halves)
                xx = wpool.tile([P, 2, EMBED_SIZE], F32, tag="xx")
                for m in range(2):
                    fc_ps = ps_fc.tile([P, EMBED_SIZE], F32, tag="fc_ps")
                    for kh in range(2):
                        nc.tensor.matmul(
                            out=fc_ps[:],
                            lhsT=smnT[kh][:, m * P : (m + 1) * P],
                            rhs=fcwt_sb[kh][:],
                            start=(kh == 0),
                            stop=(kh == 1),
                        )
                    nc.vector.tensor_add(out=xx[:, m, :], in0=fc_ps[:], in1=fcb_sb[:])
                sq3 = wpool.tile([P, 2, EMBED_SIZE], F32, tag="sq3")
                nc.vector.tensor_tensor(out=sq3[:], in0=xx[:], in1=xx[:], op=ALU.mult)
                ss3 = wpool.tile([P, 2], F32, tag="ss3")
                nc.vector.tensor_reduce(
                    out=ss3[:], in_=sq3[:], axis=mybir.AxisListType.X, op=ALU.add
                )
                nrm3 = wpool.tile([P, 2], F32, tag="nrm3")
                # n = sqrt(ss + 1e-8) + 1e-8
                nc.scalar.activation(nrm3[:], ss3[:], AF.Sqrt, bias=eps8[:, 0:1])
                nc.vector.tensor_scalar_add(nrm3[:], nrm3[:], 1e-8)
                inv3 = wpool.tile([P, 2], F32, tag="inv3")
                nc.vector.reciprocal(inv3[:], nrm3[:])
                res = wpool.tile([P, 2, EMBED_SIZE], F32, tag="res")
                inv3b = inv3[:, :, None].to_broadcast([P, 2, EMBED_SIZE])
                nc.vector.tensor_tensor(out=res[:], in0=xx[:], in1=inv3b, op=ALU.mult)
                for m in range(2):
                    nc.sync.dma_start(
                        out=out[m * P : (m + 1) * P, :], in_=res[:, m, :]
                    )

    nc.compile()
    _split_drain_waits(nc)
    return nc


# ---------------------------------------------------------------------------
# Host-side orchestration
# ---------------------------------------------------------------------------
def _phase1_prep(train_inputs, train_types, node_neigh):
    order = np.argsort(train_types, kind="stable")
    ts = train_types[order]
    tiles_s, tiles_t = [], []
    for t in range(T):
        idx_t = order[ts == t]
        if len(idx_t) == 0:
            continue
        n_tiles = -(-len(idx_t) // P)
        padded = np.concatenate(
            [idx_t, np.repeat(idx_t[-1:], n_tiles * P - len(idx_t))]
        )
        for jj in range(n_tiles):
            tiles_s.append(padded[jj * P : (jj + 1) * P])
            tiles_t.append(t)
    while len(tiles_s) % N_CORES:
        tiles_s.append(tiles_s[-1])
        tiles_t.append(tiles_t[-1])
    sample_mat = np.stack(tiles_s)  # [TT, 128]
    tile_type = np.asarray(tiles_t)
    TT = sample_mat.shape[0]
    TPC = TT // N_CORES

    flat = sample_mat.reshape(-1)
    slot_of_sample = np.zeros(B, np.int64)
    slot_of_sample[flat[::-1]] = np.arange(TT * P)[::-1]
    return sample_mat, tile_type, TPC, slot_of_sample


def _phase1_inmaps(inputs, sample_mat, tile_type, TPC):
    node_embeddings = np.asarray(inputs["node_embeddings"], np.float32)
    node_type_embeddings = np.asarray(inputs["node_type_embeddings"], np.float32)
    trans_weights = np.asarray(inputs["trans_weights"], np.float32)
    trans_weights_s1 = np.asarray(inputs["trans_weights_s1"], np.float32)
    trans_weights_s2 = np.asarray(inputs["trans_weights_s2"], np.float32)
    train_inputs = np.asarray(inputs["train_inputs"])
    node_neigh = np.asarray(inputs["node_neigh"])

    # ttab in node-block layout: [p, (t, s, u)] = ntype[s*128+p, t, u]
    npad = NBLK * P  # 2560
    ttab_pad = np.zeros((npad, T, U), np.float32)
    ttab_pad[:NUM_NODES] = node_type_embeddings
    ttab_blk = np.ascontiguousarray(
        ttab_pad.reshape(NBLK, P, T, U).transpose(1, 2, 0, 3).reshape(P, T * NBLK * U)
    ).astype(NPBF16)

    OHC = T * NBLK * P
    in_maps = []
    for k in range(N_CORES):
        smp = sample_mat[k * TPC : (k + 1) * TPC]  # [TPC, 128]
        ct = tile_type[k * TPC : (k + 1) * TPC]  # [TPC]
        ne_flat = train_inputs[smp].reshape(-1)  # order: tile-major, then p
        ne_idx = _wrap16(ne_flat)
        # one-hot neighbor counts: oh[p, ((j*T+t)*NBLK+s)*128+b]
        nn = node_neigh[smp]  # [TPC, 128, T, 10]
        s_i = nn // P
        p_i = nn % P
        jj = np.arange(TPC)[:, None, None, None]
        bb = np.arange(P)[None, :, None, None]
        tt = np.arange(T)[None, None, :, None]
        col = ((jj * T + tt) * NBLK + s_i) * P + bb
        oh_k = np.zeros((P, TPC * OHC), np.uint8)
        np.add.at(oh_k, (p_i.ravel(), col.ravel()), 1)
        s1_all = np.ascontiguousarray(
            trans_weights_s1[ct].transpose(1, 0, 2).reshape(U, TPC * DIM_A)
        ).astype(NPBF16)
        w_all = np.ascontiguousarray(
            trans_weights[ct].transpose(1, 0, 2).reshape(U, TPC * EMB)
        ).astype(NPBF16)
        s2_blk = np.ascontiguousarray(trans_weights_s2[ct][:, :, 0].T).astype(
            NPBF16
        )  # [32, TPC]
        in_maps.append(
            {
                "ttab": ttab_blk,
                "oh": oh_k.astype(NPFP8),
                "ntab": node_embeddings.astype(NPBF16),
                "neidx": ne_idx,
                "s1w": s1_all,
                "s2w": s2_blk,
                "ww": w_all,
            }
        )
    return in_maps


def _phase2_prep(region_index, region_segment_ids, slot_of_sample, NB):
    """Per-core fp8 count matrices MT[p, (k*2+m)*128+seg_l] for the dense
    segment-sum matmul, plus per-segment reciprocal counts."""
    seg_ids = np.asarray(region_segment_ids).astype(np.int64)
    new_idx = slot_of_sample[np.asarray(region_index).astype(np.int64)]
    k_i = new_idx // P
    p_i = new_idx % P
    core = seg_ids // SEG_PER_CORE
    m_i = (seg_ids % SEG_PER_CORE) // P
    sl = seg_ids % P
    col = (k_i * 2 + m_i) * P + sl
    mt = np.zeros((N_CORES, P, NB * 2 * P), np.uint8)
    np.add.at(mt, (core, p_i, col), 1)
    assert mt.max() <= 16, "fp8e4 exact-integer range exceeded"

    cnt = np.bincount(seg_ids, minlength=NSEG).astype(np.float32)
    recip_all = np.where(cnt > 0, 1.0 / np.maximum(cnt, 1.0), 0.0).astype(np.float32)

    mt_l, recip_l = [], []
    for k in range(N_CORES):
        mt_l.append(mt[k].astype(NPFP8))
        rc = recip_all[k * SEG_PER_CORE : (k + 1) * SEG_PER_CORE]
        recip_l.append(np.ascontiguousarray(rc.reshape(2, P).T))
    return mt_l, recip_l


def _phase2_inmaps(inputs, lne_cat, mt_l, recip_l):
    fc_w = np.asarray(inputs["fc_w"], np.float32)
    fc_b = np.asarray(inputs["fc_b"], np.float32)
    fcwt = np.ascontiguousarray(fc_w.T).astype(NPBF16)  # [256, 512]
    fcb = np.broadcast_to(fc_b[None, :], (P, EMBED_SIZE)).copy()
    in_maps = []
    for k in range(N_CORES):
        in_maps.append(
            {
                "lne": lne_cat,
                "mt": mt_l[k],
                "recip": recip_l[k],
                "fcwt": fcwt,
                "fcb": fcb,
            }
        )
    return in_maps


def _run_spmd_retry(nc, in_maps, retries=3, delay=45.0):
    """The axon-tunneled device occasionally reports a transient
    UNAVAILABLE/unrecoverable state; back off and retry."""
    import time as _time

    last = None
    for attempt in range(retries):
        try:
            return run_bass_kernel_spmd(nc, in_maps, list(range(N_CORES)))
        except Exception as e:  # jax.errors.JaxRuntimeError and friends
            last = e
            if attempt + 1 < retries:
                _time.sleep(delay)
    raise last


_P1_CACHE = {}
_P2_CACHE = {}


def kernel(**inputs) -> np.ndarray:
    train_inputs = np.asarray(inputs["train_inputs"])
    train_types = np.asarray(inputs["train_types"])
    node_neigh = np.asarray(inputs["node_neigh"])
    num_sms = int(inputs["num_sms"])
    max_region = int(inputs["max_region"])

    sample_mat, tile_type, TPC, slot_of_sample = _phase1_prep(
        train_inputs, train_types, node_neigh
    )
    TT = sample_mat.shape[0]

    if TPC not in _P1_CACHE:
        _P1_CACHE[TPC] = build_phase1(TPC)
    nc1 = _P1_CACHE[TPC]
    in_maps1 = _phase1_inmaps(inputs, sample_mat, tile_type, TPC)
    res1 = _run_spmd_retry(nc1, in_maps1).results

    # relay: pure concat — lne row (tile k*TPC+j, p) lives at [p, tile*256+e]
    lne_cat = np.concatenate([res1[k]["lne"] for k in range(N_CORES)], axis=1)

    NB = TT
    mt_l, recip_l = _phase2_prep(
        inputs["region_index"], inputs["region_segment_ids"], slot_of_sample, NB
    )
    if NB not in _P2_CACHE:
        _P2_CACHE[NB] = build_phase2(NB)
    nc2 = _P2_CACHE[NB]
    in_maps2 = _phase2_inmaps(inputs, lne_cat, mt_l, recip_l)
    res2 = _run_spmd_retry(nc2, in_maps2).results

    out = np.concatenate([res2[k]["out"] for k in range(N_CORES)], axis=0)
    return out.reshape(num_sms, max_region, EMBED_SIZE)
